# revision 6
# baseline (speedup 1.0000x reference)
"""Causal self-attention with RoPE for Trainium2, 8 NeuronCores.

Sharding: core c = (batch b = c//2, head-group g = c%2 of 8 heads).
Each core computes qkv for its 8 heads, RoPE, causal attention (prefix
masks derived from sorted `indices`), and a partial c_proj (its 512
input channels). Host sums the two partial c_proj outputs per batch.

Design (vs. the naive phase-separated version):
  - Software-pipelined emission: per 512-query chunk s, attention(g=0)
    is emitted first (Act builds an exp backlog), then next chunk's
    q/k matmuls + rope + transpose, remaining attention g-blocks, and
    the previous chunk's c_proj interleave into the gaps. The
    Act-engine exp stream (the phase-2 bottleneck) thus overlaps
    nearly all PE/DVE/Pool work of neighboring chunks.
  - x is transposed on the host and shipped as bf16 x^T [C, T], held
    resident in SBUF; no xT DMA-transposes on device. (fp8+DoubleRow
    qkv was tried and reverted: softmax amplifies q/k quantization
    error by |logit|, blowing the 2e-2 budget.)
  - lo-trimming (128-aligned): logits/exp/y only cover queries that
    can attend each key tile, cutting ~15% of PE + Act volume.
  - Masking is a post-exp 0/1 multiply on the Pool engine (built once
    per chunk from counts via is_gt), not a -1e9 bias matmul on PE.
  - y computed in [q, hd] orientation per 128-query subtile (moving
    operand = 65-wide v||ones, N=65 cycles instead of 512): gives
    per-partition softmax denominators, normalized with per-partition
    tensor_scalar + reciprocal (no partition_broadcast / row-move /
    partition-shift DMAs), then PE-transposed back into yTp for
    c_proj. PSUM y accumulators share banks; a single zero-writing
    warmup matmul per bank replaces per-subtile start=True (which
    would wipe the whole 2KB zero-region).
  - q/k transposes batched: one [128,1024] DMA transpose per t-tile
    into a combined qkT tensor, issued from the SP queue; Act stays
    exp-only. Weights/cos/sin/counts load order front-loads exactly
    what stage 0 needs.
"""

import numpy as np

B, T, C, H = 4, 2048, 1024, 16
HD = 64
HC = 8            # heads per core
NCORES = 8
PT = 128          # partition tile
TT = T // PT      # 16 T-tiles
QCW = 512         # q-chunk width
NQC = T // QCW    # 4
NKT = T // PT     # 16 key tiles
VW = HD + 1       # 65: v columns + ones column

_PROG_CACHE: dict = {}
_last_in_maps = None


def _build_program(sched):
    import concourse.bass as bass
    import concourse.tile as tile
    from concourse import bacc, mybir
    from concourse.masks import make_identity

    F32 = mybir.dt.float32
    F32R = mybir.dt.float32r
    BF16 = mybir.dt.bfloat16
    AT = mybir.ActivationFunctionType
    OP = mybir.AluOpType

    nc = bacc.Bacc("TRN2", target_bir_lowering=False, debug=False)

    # x^T, host-transposed: [C, T] bf16
    xt_d = nc.dram_tensor("xt", [C, T], BF16, kind="ExternalInput")
    # qkv weights, chunk-major: [128, 8 chunks, 1536]
    wqkv_d = nc.dram_tensor("wqkv", [PT, 8 * 3 * QCW], BF16,
                            kind="ExternalInput")
    wp_d = nc.dram_tensor("wp", [QCW, C], BF16, kind="ExternalInput")
    cs_d = nc.dram_tensor("csn", [2 * T, 32], F32, kind="ExternalInput")
    cntb_d = nc.dram_tensor("cntb", [1, T], F32, kind="ExternalInput")
    iota_d = nc.dram_tensor("iotas", [PT, NKT], F32, kind="ExternalInput")
    out_d = nc.dram_tensor("out", [T, C], F32, kind="ExternalOutput")

    with tile.TileContext(nc) as tc:
        with (
            tc.tile_pool(name="persist", bufs=1) as pp,
            tc.tile_pool(name="wq", bufs=1) as wqp,
            tc.tile_pool(name="work", bufs=2) as wk,
            tc.tile_pool(name="epool", bufs=6) as ep,
            tc.tile_pool(name="bpool", bufs=6) as bp,
            tc.tile_pool(name="ipool", bufs=2) as ip,
            tc.tile_pool(name="osb", bufs=2) as op_,
            tc.tile_pool(name="psA", bufs=2, space="PSUM") as psA,
            tc.tile_pool(name="psE", bufs=2, space="PSUM") as psE,
            tc.tile_pool(name="psY", bufs=2, space="PSUM") as psY,
        ):
            # ---------------- persistent tiles ----------------
            # x^T resident in SBUF: [128, chunk c (8), t (T)] bf16
            xTs = pp.tile([PT, 8 * T], BF16, tag="xTs", name="xTs")
            # combined transposed q/k store: chunk c<4 -> q group c,
            # c>=4 -> k group c-4; each chunk is [64*2 feats, T]
            qkT = pp.tile([PT, 8 * T], BF16, tag="qkT", name="qkT")
            yTp = [pp.tile([PT, T], BF16, tag=f"yTp{g}", name=f"yTp{g}") for g in range(4)]
            vaug = pp.tile([PT, NKT * HC * VW], BF16, tag="vaug")
            cs_sb = pp.tile([PT, 2 * TT * 32], F32, tag="cs")
            cntb = pp.tile([PT, T], F32, tag="cntb")
            iotas = pp.tile([PT, NKT], F32, tag="iotas")
            ident = pp.tile([PT, PT], F32, tag="ident")
            identB = pp.tile([PT, PT], BF16, tag="identB")
            z260 = pp.tile([PT, 4 * VW], BF16, tag="z260")
            ones = pp.tile([PT, 1], F32, tag="ones")

            w_sb = wqp.tile([PT, 8 * 3 * QCW], BF16, tag="w")

            # prologue DMAs, ordered so stage-0 needs land first; all on
            # the SP HWDGE queue (Pool SWDGE has ~1.5us fixed cost per DMA)
            xt_dv = xt_d[:].rearrange("(c p) t -> p c t", p=PT)
            xts_v = xTs[:].rearrange("p (c t) -> p c t", t=T)
            w_dv = wqkv_d[:].rearrange("p (c ch n) -> p ch c n", c=8, ch=3)
            w_sv = w_sb[:].rearrange("p (c ch n) -> p ch c n", c=8, ch=3)
            nc.sync.dma_start(xts_v[:, :, 0:QCW], xt_dv[:, :, 0:QCW])
            nc.sync.dma_start(w_sv[:, 0], w_dv[:, 0])  # q weights
            nc.sync.dma_start(w_sv[:, 1], w_dv[:, 1])  # k weights
            # cos|sin merged: cs_sb cols [0:512] cos, [512:1024] sin
            nc.sync.dma_start(
                cs_sb[:].rearrange("p (two t j) -> p two t j", two=2, j=32),
                cs_d[:].rearrange("(two t p) j -> p two t j", p=PT, two=2))
            nc.sync.dma_start(w_sv[:, 2], w_dv[:, 2])  # v weights
            cntb_row = pp.tile([1, T], F32, tag="cntb_row")
            nc.sync.dma_start(cntb_row[:], cntb_d[:])
            nc.gpsimd.partition_broadcast(cntb[:], cntb_row[0:1, :])
            nc.sync.dma_start(iotas[:], iota_d[:])
            wp_sb = wqp.tile([PT, (QCW // PT) * C], BF16, tag="wp")
            make_identity(nc, ident[:])
            make_identity(nc, identB[:])
            nc.vector.memset(z260[:], 0.0)
            nc.vector.memset(ones[:], 1.0)
            ones_ap = ones[:]
            ones_rep = bass.AP(ones_ap.tensor, ones_ap.offset,
                               [ones_ap.ap[0], [0, NKT], [0, HC]])
            nc.vector.tensor_copy(
                vaug[:].rearrange("p (t h c) -> p t h c", h=HC, c=VW)[:, :, :, HD],
                ones_rep,
            )

            def qT(g):
                return qkT[:, g * T:(g + 1) * T]

            def kT(g):
                return qkT[:, (4 + g) * T:(5 + g) * T]

            w_v = w_sb[:].rearrange("p (c n) -> p c n", c=8)

            # ---------------- per-tile qkv + rope ----------------
            # q,k matmuls + rope + transpose first (attention's critical
            # path); the v matmul is a separate piece that can fill PE later
            def v_tile(t):
                ps = psA.tile([PT, QCW], F32, tag="mm")
                for c in range(8):
                    nc.tensor.matmul(
                        ps[:],
                        xts_v[:, c, t * PT:(t + 1) * PT],
                        w_v[:, c, 2 * QCW:3 * QCW],
                        start=(c == 0), stop=(c == 7),
                    )
                nc.vector.tensor_copy(
                    vaug[:, t * HC * VW:(t + 1) * HC * VW]
                    .rearrange("p (h c) -> p h c", c=VW)[:, :, 0:HD],
                    ps[:].rearrange("p (h c) -> p h c", c=HD),
                )

            def qk_rope_tile(t):
                stag = wk.tile([PT, 2 * QCW], F32, tag="stag")
                for ch in range(2):  # q, k
                    ps = psA.tile([PT, QCW], F32, tag="mm")
                    for c in range(8):
                        nc.tensor.matmul(
                            ps[:],
                            xts_v[:, c, t * PT:(t + 1) * PT],
                            w_v[:, c, ch * QCW:(ch + 1) * QCW],
                            start=(c == 0), stop=(c == 7),
                        )
                    nc.vector.tensor_copy(
                        stag[:, ch * QCW:(ch + 1) * QCW], ps[:]
                    )

                # rope on q|k staging -> rot (bf16)
                rot = wk.tile([PT, 2 * QCW], BF16, tag="rot")
                sv = stag[:].rearrange("p (g two j) -> p g two j", two=2, j=32)
                rv = rot[:].rearrange("p (g two j) -> p g two j", two=2, j=32)
                X1, X2 = sv[:, :, 0, :], sv[:, :, 1, :]
                R1, R2 = rv[:, :, 0, :], rv[:, :, 1, :]
                cos_ap = cs_sb[:, t * 32:(t + 1) * 32]
                sin_ap = cs_sb[:, TT * 32 + t * 32:TT * 32 + (t + 1) * 32]
                cosr = bass.AP(cos_ap.tensor, cos_ap.offset,
                               [cos_ap.ap[0], [0, 16], [1, 32]])
                sinr = bass.AP(sin_ap.tensor, sin_ap.offset,
                               [sin_ap.ap[0], [0, 16], [1, 32]])
                t1 = wk.tile([PT, QCW], F32, tag="tmp1", bufs=1)
                t2 = wk.tile([PT, QCW], F32, tag="tmp2", bufs=1)
                t1v = t1[:].rearrange("p (g j) -> p g j", j=32)
                t2v = t2[:].rearrange("p (g j) -> p g j", j=32)
                nc.vector.tensor_tensor(t1v, X1, cosr, OP.mult)
                nc.vector.tensor_tensor(t2v, X2, sinr, OP.mult)
                nc.vector.tensor_tensor(R1, t1v, t2v, OP.subtract)
                t3 = wk.tile([PT, QCW], F32, tag="tmp1", bufs=1)
                t4 = wk.tile([PT, QCW], F32, tag="tmp2", bufs=1)
                t3v = t3[:].rearrange("p (g j) -> p g j", j=32)
                t4v = t4[:].rearrange("p (g j) -> p g j", j=32)
                nc.gpsimd.tensor_tensor(t3v, X1, sinr, OP.mult)
                nc.gpsimd.tensor_tensor(t4v, X2, cosr, OP.mult)
                nc.gpsimd.tensor_tensor(R2, t3v, t4v, OP.add)

                # one batched transpose: rot [128, 8*128] -> 8 chunks of qkT
                nc.sync.dma_start(
                    qkT[:].rearrange("p (g tt) -> p g tt", tt=T)
                    [:, :, t * PT:(t + 1) * PT],
                    rot[:], transpose=True,
                )

            # ---------------- attention for one q-chunk ----------------
            def build_bts(J):
                bts = {}
                for (i, lo, hi) in sched[J]:
                    if hi > lo:
                        bt = bp.tile([PT, QCW], BF16, tag="B")
                        # 1 where key k is VALID for query q (k < count[q])
                        nc.vector.tensor_scalar(
                            bt[:, 0:hi - lo],
                            cntb[:, J * QCW + lo:J * QCW + hi],
                            iotas[:, i:i + 1], None, OP.is_gt,
                        )
                        bts[i] = bt
                return bts

            def attention_g(J, g, bts):
                qs = slice(J * QCW, (J + 1) * QCW)
                # last key tile touching each 128-query subtile
                lastj = [max(i for (i, lo, hi) in sched[J]
                             if lo < PT * (j + 1)) for j in range(4)]
                # y in [q, hd] orientation: yq[hh] = [128 q, 4 subtiles x 65]
                # (64 v-cols + denominator col per subtile)
                yqs = [psY.tile([PT, 4 * VW], F32, tag="yq", name="yq0"),
                       psY.tile([PT, 4 * VW], F32, tag="yq", name="yq1")]
                # zero-init both accumulator banks with one explicit
                # whole-range matmul each: later subtile matmuls all use
                # start=False (a start=True per subtile would mark the whole
                # 2KB bank pending-zero and wipe sibling subtiles)
                for hh in (0, 1):
                    nc.tensor.matmul(
                        yqs[hh][:], identB[:], z260[:],
                        start=True, stop=False, skip_group_check=True,
                    )
                for (i, lo, hi) in sched[J]:
                    ks = slice(i * PT, (i + 1) * PT)
                    qr = slice(J * QCW + lo, (J + 1) * QCW)
                    bnd = i in bts
                    # both heads' logits into one 2-bank PSUM pair so a
                    # single dual-range exp covers them (halves Act op count)
                    et = psE.tile([PT, 2 * QCW], F32, tag="et")
                    e_sb = ep.tile([PT, 2 * QCW], BF16, tag="E")
                    for hh in (0, 1):
                        nc.tensor.matmul(
                            et[:, hh * QCW + lo:(hh + 1) * QCW],
                            kT(g)[64 * hh:64 * hh + HD, ks],
                            qT(g)[64 * hh:64 * hh + HD, qr],
                            start=True, stop=True,
                        )
                    etv = et[:].rearrange("p (two q) -> p two q", two=2)
                    ev = e_sb[:].rearrange("p (two q) -> p two q", two=2)
                    nc.scalar.activation(
                        ev[:, :, lo:QCW], etv[:, :, lo:QCW], AT.Exp,
                        scale=0.125,
                    )
                    if bnd:
                        # zero masked entries post-exp (0/1 multiply on
                        # Pool), bts broadcast across the head pair
                        bt_ap = bts[i][:, 0:hi - lo]
                        bt2 = bass.AP(bt_ap.tensor, bt_ap.offset,
                                      [bt_ap.ap[0], [0, 2], [1, hi - lo]])
                        nc.gpsimd.tensor_tensor(
                            ev[:, :, lo:hi], ev[:, :, lo:hi], bt2, OP.mult,
                        )
                    for hh in (0, 1):
                        h = 2 * g + hh
                        vcol = i * HC * VW + h * VW
                        for j in range(lo // PT, 4):
                            nc.tensor.matmul(
                                yqs[hh][:, j * VW:(j + 1) * VW],
                                e_sb[:, hh * QCW + j * PT:hh * QCW + (j + 1) * PT],
                                vaug[:, vcol:vcol + VW],
                                start=False, stop=(i == lastj[j]),
                                skip_group_check=True,
                            )
                # normalize with per-partition scalars, then PE-transpose
                # each [128q, 128feat] block into yTp[g]; the transpose
                # target shares the psE ring (bank-granular slots)
                ytp = psY.tile([PT, QCW], BF16, tag="yq", name="ytp")
                invs = []
                for hh in (0, 1):
                    invS = ip.tile([PT, 4], F32, tag="invS")
                    nc.vector.reciprocal(
                        invS[:],
                        yqs[hh][:].rearrange("p (j c) -> p j c", c=VW)[:, :, HD],
                    )
                    invs.append(invS)
                for j in range(4):
                    ysc = wk.tile([PT, PT], BF16, tag="ysc", bufs=8)
                    for hh in (0, 1):
                        nc.vector.tensor_scalar(
                            ysc[:, hh * 64:hh * 64 + 64],
                            yqs[hh][:, j * VW:j * VW + HD],
                            invs[hh][:, j:j + 1], None, OP.mult,
                        )
                    nc.tensor.matmul(
                        ytp[:, j * PT:(j + 1) * PT], ysc[:], identB[:],
                        is_transpose=True,
                    )
                nc.vector.tensor_copy(yTp[g][:, qs], ytp[:])

            # ---------------- c_proj for one chunk's t-tiles -------------
            def cproj_chunk(J, half=None):
                ts = range(4 * J, 4 * J + 4)
                if half == 0:
                    ts = range(4 * J, 4 * J + 2)
                elif half == 1:
                    ts = range(4 * J + 2, 4 * J + 4)
                for t in ts:
                    for n in range(C // QCW):
                        ps = psA.tile([PT, QCW], F32, tag="mm")
                        for k4 in range(QCW // PT):
                            nc.tensor.matmul(
                                ps[:],
                                yTp[k4][:, t * PT:(t + 1) * PT],
                                wp_sb[:, k4 * C + n * QCW: k4 * C + (n + 1) * QCW],
                                start=(k4 == 0), stop=(k4 == QCW // PT - 1),
                            )
                        o_sb = op_.tile([PT, QCW], F32, tag="osb")
                        nc.vector.tensor_copy(o_sb[:], ps[:])
                        nc.sync.dma_start(
                            out_d[t * PT:(t + 1) * PT, n * QCW:(n + 1) * QCW], o_sb[:]
                        )

            # ---------------- interleaved schedule ----------------
            # qkv runs one stage ahead so rope+transpose for chunk s+1
            # overlap attention(s)'s Act-bound exp stream. attention g=0
            # is emitted before the qkv/cproj PE detour so Act builds a
            # backlog that covers it; masks (bts) build before next-stage
            # rope so attention is never stuck behind it on DVE.
            # stage 0 prerequisites: q,k for tiles 0-3 (v can lag slightly)
            for t in range(0, 4):
                qk_rope_tile(t)
            # deferred loads: needed from stage-0's lookahead / cproj(0) on,
            # but must not delay the first qkT transposes on the SP queue
            nc.sync.dma_start(xts_v[:, :, QCW:2 * QCW],
                              xt_dv[:, :, QCW:2 * QCW])
            nc.sync.dma_start(wp_sb[:].rearrange("p (k n) -> p k n", n=C),
                              wp_d[:].rearrange("(k p) n -> p k n", p=PT))
            for t in range(0, 4):
                v_tile(t)
            for s in range(NQC):
                bts = build_bts(s)
                nxt = [] if s + 1 >= NQC else list(range(4 * s + 4, 4 * s + 8))
                attention_g(s, 0, bts)
                if nxt:
                    qk_rope_tile(nxt[0])
                    qk_rope_tile(nxt[1])
                if s + 2 < NQC:
                    cs2 = slice((s + 2) * QCW, (s + 3) * QCW)
                    nc.sync.dma_start(xts_v[:, :, cs2], xt_dv[:, :, cs2])
                attention_g(s, 1, bts)
                if nxt:
                    qk_rope_tile(nxt[2])
                    qk_rope_tile(nxt[3])
                if s > 0:
                    cproj_chunk(s - 1, half=0)
                attention_g(s, 2, bts)
                if nxt:
                    v_tile(nxt[0])
                    v_tile(nxt[1])
                if s > 0:
                    cproj_chunk(s - 1, half=1)
                attention_g(s, 3, bts)
                if nxt:
                    v_tile(nxt[2])
                    v_tile(nxt[3])
            cproj_chunk(NQC - 1)

    nc.compile()
    return nc


def _get_program(sched):
    key = tuple(tuple(t) for t in sched)
    if key not in _PROG_CACHE:
        _PROG_CACHE[key] = _build_program(sched)
    return _PROG_CACHE[key]


def _prep(x, W_attn, W_proj, indices):
    half = HD // 2
    inv_freq = (1.0 / (10000.0 ** (np.arange(half, dtype=np.float32)
                                   / np.float32(half)))).astype(np.float32)

    counts = np.empty((B, T), np.int64)
    for b in range(B):
        counts[b] = np.searchsorted(indices[b], indices[b], side="right")

    sched = []
    for J in range(NQC):
        chunks = counts[:, J * QCW:(J + 1) * QCW]
        km = int((chunks.max() + PT - 1) // PT)
        tiles = []
        for i in range(km):
            lo = min(int(np.searchsorted(chunks[b], PT * i, side="right"))
                     for b in range(B))
            hi = max(int(np.searchsorted(chunks[b], PT * (i + 1) - 1,
                                         side="right"))
                     for b in range(B))
            # floor lo to a 128 multiple: the flipped y-matmul consumes E in
            # full 128-query subtiles, and the mask zeroes [lo128, lo)
            lo = (lo // PT) * PT
            if lo < QCW:
                tiles.append((i, lo, min(hi, QCW)))
        sched.append(tiles)

    iotas = (np.arange(PT, dtype=np.float32)[:, None]
             + PT * np.arange(NKT, dtype=np.float32)[None, :]).copy()

    import ml_dtypes
    BF16 = ml_dtypes.bfloat16

    in_maps = []
    for core in range(NCORES):
        b, g = core // 2, core % 2
        wq = W_attn[:, g * QCW:(g + 1) * QCW]
        wk_ = W_attn[:, C + g * QCW: C + (g + 1) * QCW]
        wv = W_attn[:, 2 * C + g * QCW: 2 * C + (g + 1) * QCW]
        wqkv = np.concatenate([wq, wk_, wv], axis=1)  # [C, 1536]
        # chunk-major: [p, c, n] = wqkv[128*c + p, n]
        wcm = np.ascontiguousarray(
            wqkv.reshape(8, PT, 3 * QCW).transpose(1, 0, 2)
            .reshape(PT, 8 * 3 * QCW)
        ).astype(BF16)
        wp = np.ascontiguousarray(
            W_proj[g * QCW:(g + 1) * QCW, :]).astype(BF16)
        xt = np.ascontiguousarray(x[b].T).astype(BF16)
        ang = indices[b].astype(np.float32)[:, None] * inv_freq[None, :]
        csn = np.concatenate([np.cos(ang), np.sin(ang)], axis=0)
        in_maps.append({
            "xt": xt,
            "wqkv": wcm,
            "wp": wp,
            "csn": np.ascontiguousarray(csn).astype(np.float32),
            "cntb": counts[b].astype(np.float32)[None, :].copy(),
            "iotas": iotas,
        })
    return sched, in_maps


def kernel(x, W_attn, W_proj, indices):
    global _last_in_maps
    x = np.asarray(x, dtype=np.float32)
    W_attn = np.asarray(W_attn, dtype=np.float32)
    W_proj = np.asarray(W_proj, dtype=np.float32)
    indices = np.asarray(indices)

    sched, in_maps = _prep(x, W_attn, W_proj, indices)
    _last_in_maps = in_maps
    nc = _get_program(sched)

    from concourse.bass_utils import run_bass_kernel_spmd
    res = run_bass_kernel_spmd(nc, in_maps, list(range(NCORES)))

    out = np.empty((B, T, C), np.float32)
    for b in range(B):
        out[b] = res.results[2 * b]["out"] + res.results[2 * b + 1]["out"]
    return out
